# revision 1
# baseline (speedup 1.0000x reference)
"""CapsuleLayer dynamic-routing kernel for 8 Trainium2 NeuronCores.

Problem: x [128, 2048, 8], W [32, 2048, 16, 8] (fp32)
  u_hat[b,j,i,d] = sum_p W[j,i,d,p] * x[b,i,p]
  3 rounds of routing-by-agreement (softmax over j, squash), no
  persistent logits needed: b_k = (sum_{m<k} out_m) . u_hat, so each
  round is a streaming pass over i needing only O_k = sum out_m.

Sharding: i (input capsules) split 8 ways; every core holds the full
batch B=128 on SBUF partitions. Per-round partial sums s[b,(j,d)]
(256KB/core) are reduced on the host between the three launches.
"""

import numpy as np
from contextlib import ExitStack

import concourse.bass as bass
import concourse.mybir as mybir
from concourse import tile
from concourse.bass_utils import run_bass_kernel_spmd

# ---------------------------------------------------------------------------
# Shapes (hardcoded for this problem)
B, I, P = 128, 2048, 8
J, D = 32, 16
JD = J * D               # 512
N_CORES = 8
I_LOC = I // N_CORES     # 256
EPS = 1e-7
GROUP = 4                # i's per routing group (psum tile = GROUP banks)
N_GROUPS = I_LOC // GROUP

_f32 = mybir.dt.float32


# ---------------------------------------------------------------------------
# Walrus compat: this toolchain rejects sync waits on InstDrain and >2 on
# InstEventSemaphore. Emit the waits as standalone nops before the drain.
def _apply_tile_compat():
    from concourse.vector_clock import ScopedClock

    def _strip_waits(inst):
        si = inst.sync_info
        if not si or not si.on_wait:
            return []
        waits = list(si.on_wait)
        si.on_wait = []
        inst.sync_info = si
        return waits

    def _nop_with_wait(eng, w):
        nop = eng.nop(nofuse=True, hint="drain_wait_split")
        nsi = nop.ins.sync_info
        if nsi is None:
            nsi = mybir.SyncInfo(on_wait=[], on_update=[])
        nsi.on_wait = list(nsi.on_wait or []) + [w]
        nop.ins.sync_info = nsi

    def _patched_multi_engine_barrier(self, engines):
        for inst in bass._bass_rust._multi_engine_barrier_insts(
            self, list(engines)
        ):
            eng = self.engines[inst.engine]
            for w in _strip_waits(inst):
                _nop_with_wait(eng, w)
            eng.add_instruction(inst)

    def _patched_drain_and_barrier(self, tick_clock, wait_clock):
        nop_inst = self.nc.sync.nop(nofuse=True, hint="drain_wait_split")
        wait_clock.add_sem_waits(
            nop_inst.ins, ScopedClock({None: tick_clock.global_clock})
        )
        si = nop_inst.ins.sync_info
        if si and si.on_wait and len(si.on_wait) > 1:
            extra = list(si.on_wait[1:])
            si.on_wait = [si.on_wait[0]]
            nop_inst.ins.sync_info = si
            for w in extra:
                _nop_with_wait(self.nc.sync, w)
        self.nc.sync.drain()

        self.nc.all_engine_barrier()
        assert self.sems is not None
        popped = self.nc._tile_sem_poison_stack.pop()
        assert popped is self._sem_poison
        self.nc.clear_and_free_semaphores(list(self.sems.allocated().values()))
        # No trailing all_engine_barrier: every engine is already past the
        # pre-clear barrier (done touching semaphores), nothing reads them
        # afterwards, and NEFF completion only needs each engine to halt.

    # Scheduled body instructions can also end up with >1 wait (e.g. a
    # matmul waiting on two DMAs). Spill extras onto same-engine NoOps
    # inserted immediately before the instruction.
    _WAIT_CAPS = {"InstDrain": 0, "InstEventSemaphore": 2}
    _orig_add_instruction = tile.TileContext._add_instruction

    def _patched_add_instruction(self, inst):
        si = inst.sync_info
        cap = _WAIT_CAPS.get(type(inst).__name__, 1)
        if si and si.on_wait and len(si.on_wait) > cap:
            waits = list(si.on_wait)
            si.on_wait = waits[:cap]
            inst.sync_info = si
            for w in waits[cap:]:
                nop = mybir.InstNoOp(
                    name=f"I-{self.nc.next_id()}-waitspill", ins=[], outs=[]
                )
                nop.engine = inst.engine
                nop.sync_info = mybir.SyncInfo(on_wait=[w], on_update=[])
                _orig_add_instruction(self, nop)
        _orig_add_instruction(self, inst)

    bass.Bass.multi_engine_barrier = _patched_multi_engine_barrier
    tile.TileContext._drain_and_barrier = _patched_drain_and_barrier
    tile.TileContext._add_instruction = _patched_add_instruction


_apply_tile_compat()


# ---------------------------------------------------------------------------
# Launch 1: s0_part[b,(j,d)] = sum_{i local} u_hat[b,j,i,d]
# (iteration 0 has exactly uniform c = 1/32, applied on the host)
def build_l1():
    nc = bass.Bass("TRN2", target_bir_lowering=False, debug=False)
    n_chunks = (I_LOC * P) // 128  # 16
    xw1 = nc.dram_tensor(
        "xw1", [n_chunks, 128, B + JD], _f32, kind="ExternalInput").ap()
    sp = nc.dram_tensor("sp", [B, JD], _f32, kind="ExternalOutput").ap()
    with ExitStack() as ctx:
        tc = ctx.enter_context(tile.TileContext(nc))
        xpool = ctx.enter_context(tc.tile_pool(name="xw1", bufs=4))
        ppool = ctx.enter_context(tc.tile_pool(name="ps", bufs=1, space="PSUM"))
        opool = ctx.enter_context(tc.tile_pool(name="o", bufs=1))
        psum = ppool.tile([B, JD], _f32)
        for q in range(n_chunks):
            t = xpool.tile([128, B + JD], _f32)
            nc.sync.dma_start(t[:], xw1[q])
            nc.tensor.matmul(
                psum[:], lhsT=t[:, :B], rhs=t[:, B:],
                start=(q == 0), stop=(q == n_chunks - 1),
            )
        out = opool.tile([B, JD], _f32)
        nc.scalar.copy(out[:], psum[:])
        nc.sync.dma_start(sp[:], out[:])
    return nc


# ---------------------------------------------------------------------------
# Launches 2 & 3: one routing round.
#   g[b,j,i]  = sum_d O[b,j,d] * u_hat[b,j,i,d]
#   c         = softmax_j(g)
#   s_part    = sum_{i local} c * u_hat
#
# x and W arrive interleaved in 16-i blocks ("xw": per block, the x
# slab [P, 16*B] then the W slab [P, 16*JD], both p-major) so each
# block is one large DMA instead of 16 small ones.
BLK = 16                       # i's per DMA block
N_BLKS = I_LOC // BLK          # 16
XW_X = BLK * B                 # 2048 x columns per block
XW_W = BLK * JD                # 8192 W columns per block
XW_COLS = XW_X + XW_W          # 10240


def build_l2():
    nc = bass.Bass("TRN2", target_bir_lowering=False, debug=False)
    xw = nc.dram_tensor(
        "xw", [N_BLKS, P, XW_COLS], _f32, kind="ExternalInput").ap()
    x2d = nc.dram_tensor("x2", [B, I_LOC * P], _f32, kind="ExternalInput").ap()
    w2d = nc.dram_tensor(
        "w2", [J, D, I_LOC * P], _f32, kind="ExternalInput").ap()
    otd = nc.dram_tensor("ot", [D, J * B], _f32, kind="ExternalInput").ap()
    sp = nc.dram_tensor("sp", [B, JD], _f32, kind="ExternalOutput").ap()

    IP = I_LOC * P  # 2048

    with ExitStack() as ctx:
        tc = ctx.enter_context(tile.TileContext(nc))
        wpool = ctx.enter_context(tc.tile_pool(name="xw", bufs=2))
        tpool = ctx.enter_context(tc.tile_pool(name="tmp", bufs=2))
        gpool = ctx.enter_context(tc.tile_pool(name="g", bufs=2))
        bpool = ctx.enter_context(tc.tile_pool(name="big", bufs=1))
        apool = ctx.enter_context(tc.tile_pool(name="acc", bufs=1))

        # wide accumulators: one GROUP-lane per i-position, reduced once
        # at the end. Two of them so even groups accumulate on DVE and
        # odd groups on GpSimd, halving the DVE add chain.
        s_wide = apool.tile([B, GROUP * JD], _f32)
        nc.gpsimd.memset(s_wide[:], 0.0)
        s_wide2 = apool.tile([B, GROUP * JD], _f32)
        nc.gpsimd.memset(s_wide2[:], 0.0)

        # ---- phase A: g[b,(j,i)] = sum_p x2[b,(i,p)] * (O_j @ W2_j)[b,(i,p)]
        # g_all is reused in place for e = exp(g - m) and then c (softmax
        # numerator / weights): every op is elementwise with identical
        # input/output traversal order.
        g_all = bpool.tile([B, J * I_LOC], _f32)

        with tc.tile_pool(name="vps", bufs=2, space="PSUM") as vppool, \
             tc.tile_pool(name="pa", bufs=1) as papool, \
             tc.tile_pool(name="w2", bufs=2) as w2pool:
            x2 = papool.tile([B, IP], _f32)
            nc.sync.dma_start(x2[:], x2d[:])
            ot = papool.tile([D, J * B], _f32)
            nc.sync.dma_start(ot[:], otd[:])
            for j in range(J):
                w2t = w2pool.tile([D, IP], _f32)
                nc.sync.dma_start(w2t[:], w2d[j])
                vps = vppool.tile([B, IP], _f32)
                for q in range(IP // 512):
                    nc.tensor.matmul(
                        vps[:, q * 512:(q + 1) * 512],
                        lhsT=ot[:, j * B:(j + 1) * B],
                        rhs=w2t[:, q * 512:(q + 1) * 512],
                        start=True, stop=True,
                    )
                xv = tpool.tile([B, IP], _f32)
                nc.vector.tensor_tensor(
                    xv[:], x2[:], vps[:], op=mybir.AluOpType.mult,
                )
                nc.vector.reduce_sum(
                    g_all[:, j * I_LOC:(j + 1) * I_LOC],
                    xv[:].rearrange("b (i p) -> b i p", i=I_LOC, p=P),
                    axis=mybir.AxisListType.X,
                )

        # ---- softmax over j (free-dim strided, one shot for all i).
        # No max-subtraction: g = O.u_hat with squashed O (|O_j| < 1) is
        # bounded well inside exp's fp32 range, and softmax is shift-
        # invariant, so exp(g)/sum exp(g) matches the reference exactly.
        gjv = g_all[:].rearrange("b (j i) -> b j i", j=J, i=I_LOC)
        giv = g_all[:].rearrange("b (j i) -> b i j", j=J, i=I_LOC)
        nc.scalar.activation(
            g_all[:], g_all[:], mybir.ActivationFunctionType.Exp
        )
        Z = bpool.tile([B, I_LOC], _f32)
        nc.vector.reduce_sum(Z[:], giv, axis=mybir.AxisListType.X)
        Zr = bpool.tile([B, I_LOC], _f32)
        nc.vector.reciprocal(Zr[:], Z[:])
        nc.vector.tensor_tensor(
            gjv, gjv, Zr[:].unsqueeze(1).broadcast_to([B, J, I_LOC]),
            op=mybir.AluOpType.mult,
        )
        c_v = giv

        # ---- phase B: s += sum_i c * u_hat, u_hat recomputed per group.
        # The weighted tiles w are accumulated on the PE into a resident
        # PSUM region via identity matmuls (start=False), so the DVE only
        # does the c-multiply.
        ppool = ctx.enter_context(tc.tile_pool(name="ps", bufs=2, space="PSUM"))
        xw_tiles = {}
        for gi in range(N_GROUPS):
            blk, sub = divmod(gi * GROUP, BLK)
            if sub == 0:
                xwt = wpool.tile([P, XW_COLS], _f32)
                nc.sync.dma_start(xwt[:], xw[blk])
                xw_tiles[blk] = xwt
            xwt = xw_tiles[blk]
            psum = ppool.tile([B, GROUP * JD], _f32)
            for t in range(GROUP):
                ib = sub + t           # i index within the block
                nc.tensor.matmul(
                    psum[:, t * JD:(t + 1) * JD],
                    lhsT=xwt[:, ib * B:(ib + 1) * B],
                    rhs=xwt[:, XW_X + ib * JD:XW_X + (ib + 1) * JD],
                    start=True, stop=True,
                )
            pv = psum[:].rearrange("b (i j d) -> b i j d", i=GROUP, j=J, d=D)
            cslice = c_v[:, gi * GROUP:(gi + 1) * GROUP, :]
            w = tpool.tile([B, GROUP * JD], _f32)
            wv = w[:].rearrange("b (i j d) -> b i j d", i=GROUP, j=J, d=D)
            nc.vector.tensor_tensor(
                wv, pv, cslice.unsqueeze(3).broadcast_to([B, GROUP, J, D]),
                op=mybir.AluOpType.mult,
            )
            if gi % 2 == 0:
                nc.vector.tensor_add(s_wide[:], s_wide[:], w[:])
            else:
                nc.gpsimd.tensor_add(s_wide2[:], s_wide2[:], w[:])

        nc.vector.tensor_add(s_wide[:], s_wide[:], s_wide2[:])
        s_acc = gpool.tile([B, JD], _f32)
        nc.vector.reduce_sum(
            s_acc[:],
            s_wide[:].rearrange("b (i jd) -> b jd i", i=GROUP, jd=JD),
            axis=mybir.AxisListType.X,
        )
        nc.sync.dma_start(sp[:], s_acc[:])
    return nc


# ---------------------------------------------------------------------------
# Host glue
def _squash(s):
    v = s.reshape(B, J, D)
    s2 = np.sum(np.square(v), axis=-1, keepdims=True)
    scale = s2 / (1.0 + s2) / np.sqrt(s2 + EPS)
    return (scale * v).astype(np.float32)


_cache = {}


def _get_nc(name):
    if name not in _cache:
        _cache[name] = build_l1() if name == "l1" else build_l2()
    return _cache[name]


def _prep_inputs(x, W):
    """Per-core host-side re-layouts (all fp32, cheap transposes)."""
    per_core = []
    for c in range(N_CORES):
        sl = slice(c * I_LOC, (c + 1) * I_LOC)
        xc = x[:, sl, :]                                   # [B, I_LOC, P]
        wc = W[:, sl, :, :]                                # [J, I_LOC, D, P]
        xp = np.ascontiguousarray(
            xc.transpose(1, 2, 0).reshape(I_LOC * P, B))   # (i,p),b
        wt = np.ascontiguousarray(
            wc.transpose(1, 3, 0, 2).reshape(I_LOC * P, JD))  # (i,p),(j,d)
        # L1: interleave x/W per 128-row chunk so each chunk is one DMA
        n_chunks = (I_LOC * P) // 128
        xw1 = np.empty((n_chunks, 128, B + JD), np.float32)
        xw1[:, :, :B] = xp.reshape(n_chunks, 128, B)
        xw1[:, :, B:] = wt.reshape(n_chunks, 128, JD)
        # interleaved blocks for L2/L3: per 16-i block, [P, 16*B | 16*JD]
        xb = xc.transpose(2, 1, 0).reshape(P, N_BLKS, BLK * B)  # p,(blk,i*b)
        wb = wc.transpose(3, 1, 0, 2).reshape(P, N_BLKS, BLK, JD)
        xw = np.empty((N_BLKS, P, XW_COLS), np.float32)
        xw[:, :, :XW_X] = xb.transpose(1, 0, 2)
        xw[:, :, XW_X:] = wb.transpose(1, 0, 2, 3).reshape(N_BLKS, P, XW_W)
        # V-trick layouts
        x2 = np.ascontiguousarray(xc.reshape(B, I_LOC * P))      # b,(i,p)
        w2 = np.ascontiguousarray(
            wc.transpose(0, 2, 1, 3).reshape(J, D, I_LOC * P))   # j,d,(i,p)
        per_core.append({"xw1": xw1, "xw": xw, "x2": x2, "w2": w2})
    return per_core


def _ot_layout(O):
    """O [B, JD] -> lhsT layout [D, J*B] for the V matmuls."""
    return np.ascontiguousarray(
        O.reshape(B, J, D).transpose(2, 1, 0).reshape(D, J * B))


def _run(nc, in_maps, **kw):
    res = run_bass_kernel_spmd(nc, in_maps, list(range(N_CORES)), **kw)
    return res


def kernel(x, W, _collect_times=None):
    x = np.asarray(x, dtype=np.float32)
    W = np.asarray(W, dtype=np.float32)
    pc = _prep_inputs(x, W)

    nc1 = _get_nc("l1")
    nc2 = _get_nc("l2")

    r1 = _run(nc1, [{"xw1": p["xw1"]} for p in pc])
    s0 = np.sum([r1.results[c]["sp"] for c in range(N_CORES)], axis=0)
    s0 *= (1.0 / J)
    out0 = _squash(s0)
    O1 = out0.reshape(B, JD)

    ot1 = _ot_layout(O1)
    r2 = _run(nc2, [
        {"xw": p["xw"], "x2": p["x2"], "w2": p["w2"], "ot": ot1}
        for p in pc
    ])
    s1 = np.sum([r2.results[c]["sp"] for c in range(N_CORES)], axis=0)
    out1 = _squash(s1)
    O2 = (out0 + out1).reshape(B, JD)

    ot2 = _ot_layout(O2)
    r3 = _run(nc2, [
        {"xw": p["xw"], "x2": p["x2"], "w2": p["w2"], "ot": ot2}
        for p in pc
    ])
    s2 = np.sum([r3.results[c]["sp"] for c in range(N_CORES)], axis=0)
    out2 = _squash(s2)

    if _collect_times is not None:
        for r in (r1, r2, r3):
            _collect_times.append(r.exec_time_ns)
    return out2



# revision 2
# speedup vs baseline: 1.0030x; 1.0030x over previous
"""CapsuleLayer dynamic-routing kernel v2 for 8 Trainium2 NeuronCores.

Strategy vs v1: move every reduction/replication onto the PE (whose cost
is only proportional to output columns) and keep just the two irreducible
broadcast-multiplies per (j, round) on DVE/GpSimd, in bf16.

Layout (per core, I_LOC = 256): i = ih*16 + il, SBUF partitions
q = p*16 + il. Then per (j, ih) everything is 128x128 tiles:
  V-matmul:  V[q, b]   = sum_d Wv[d, q] * O^T[d, b]        (PE)
  xV mult:   xV = x_t (.) V                                 (DVE/Pool)
  g-reduce:  g[b, il]  = xV^T @ E16   (sums over p)         (PE)
  exp:       e = exp(g)                                     (ACT, from PSUM)
  softmax Z: strided reduce over j + reciprocal             (DVE)
  replicate: w[q, b] = R_v^T @ e_t    (broadcast over p)    (PE)
  y mult:    y = xz (.) w   (xz = x * 1/Z, replicated)      (DVE/Pool)
  s-matmul:  s^T[d, b] += Ws^T @ y   (accum over ih)        (PE)

3 launches (round 0 uniform-c + two routing rounds); the tiny
squash/reduce between launches runs on the host, as in v1.
"""

import numpy as np
import ml_dtypes
from contextlib import ExitStack

import concourse.bass as bass
import concourse.mybir as mybir
from concourse import tile
from concourse.bass_utils import run_bass_kernel_spmd

BF16 = ml_dtypes.bfloat16

# ---------------------------------------------------------------------------
B, I, P = 128, 2048, 8
J, D = 32, 16
JD = J * D               # 512
N_CORES = 8
I_LOC = I // N_CORES     # 256
IH = IL = 16
EPS = 1e-7

_f32 = mybir.dt.float32
_bf16 = mybir.dt.bfloat16


# ---------------------------------------------------------------------------
# Walrus compat: this toolchain rejects sync waits on InstDrain and >2 on
# InstEventSemaphore. Emit the waits as standalone nops before the drain.
def _apply_tile_compat():
    from concourse.vector_clock import ScopedClock

    def _strip_waits(inst):
        si = inst.sync_info
        if not si or not si.on_wait:
            return []
        waits = list(si.on_wait)
        si.on_wait = []
        inst.sync_info = si
        return waits

    def _nop_with_wait(eng, w):
        nop = eng.nop(nofuse=True, hint="drain_wait_split")
        nsi = nop.ins.sync_info
        if nsi is None:
            nsi = mybir.SyncInfo(on_wait=[], on_update=[])
        nsi.on_wait = list(nsi.on_wait or []) + [w]
        nop.ins.sync_info = nsi

    def _patched_multi_engine_barrier(self, engines):
        for inst in bass._bass_rust._multi_engine_barrier_insts(
            self, list(engines)
        ):
            eng = self.engines[inst.engine]
            for w in _strip_waits(inst):
                _nop_with_wait(eng, w)
            eng.add_instruction(inst)

    def _patched_drain_and_barrier(self, tick_clock, wait_clock):
        nop_inst = self.nc.sync.nop(nofuse=True, hint="drain_wait_split")
        wait_clock.add_sem_waits(
            nop_inst.ins, ScopedClock({None: tick_clock.global_clock})
        )
        si = nop_inst.ins.sync_info
        if si and si.on_wait and len(si.on_wait) > 1:
            extra = list(si.on_wait[1:])
            si.on_wait = [si.on_wait[0]]
            nop_inst.ins.sync_info = si
            for w in extra:
                _nop_with_wait(self.nc.sync, w)
        self.nc.sync.drain()

        self.nc.all_engine_barrier()
        assert self.sems is not None
        popped = self.nc._tile_sem_poison_stack.pop()
        assert popped is self._sem_poison
        self.nc.clear_and_free_semaphores(list(self.sems.allocated().values()))

    _WAIT_CAPS = {"InstDrain": 0, "InstEventSemaphore": 2}
    _orig_add_instruction = tile.TileContext._add_instruction

    def _patched_add_instruction(self, inst):
        si = inst.sync_info
        cap = _WAIT_CAPS.get(type(inst).__name__, 1)
        if si and si.on_wait and len(si.on_wait) > cap:
            waits = list(si.on_wait)
            si.on_wait = waits[:cap]
            inst.sync_info = si
            for w in waits[cap:]:
                nop = mybir.InstNoOp(
                    name=f"I-{self.nc.next_id()}-waitspill", ins=[], outs=[]
                )
                nop.engine = inst.engine
                nop.sync_info = mybir.SyncInfo(on_wait=[w], on_update=[])
                _orig_add_instruction(self, nop)
        _orig_add_instruction(self, inst)

    bass.Bass.multi_engine_barrier = _patched_multi_engine_barrier
    tile.TileContext._drain_and_barrier = _patched_drain_and_barrier
    tile.TileContext._add_instruction = _patched_add_instruction


_apply_tile_compat()


# ---------------------------------------------------------------------------
# Launch 1: s0_part[b,(j,d)] = sum_{i local} u_hat[b,j,i,d], bf16 inputs.
def build_l1():
    nc = bass.Bass("TRN2", target_bir_lowering=False, debug=False)
    n_chunks = (I_LOC * P) // 128  # 16
    xw1 = nc.dram_tensor(
        "xw1", [n_chunks, 128, B + JD], _bf16, kind="ExternalInput").ap()
    sp = nc.dram_tensor("sp", [B, JD], _bf16, kind="ExternalOutput").ap()
    with ExitStack() as ctx:
        tc = ctx.enter_context(tile.TileContext(nc))
        xpool = ctx.enter_context(tc.tile_pool(name="xw1", bufs=1))
        ppool = ctx.enter_context(tc.tile_pool(name="ps", bufs=1, space="PSUM"))
        opool = ctx.enter_context(tc.tile_pool(name="o", bufs=1))
        psum = ppool.tile([B, JD], _f32)
        big = xpool.tile([128, n_chunks * (B + JD)], _bf16)
        bv = big[:].rearrange("p (c x) -> p c x", c=n_chunks, x=B + JD)
        # eighth-loads so the first matmuls start early
        for q in range(8):
            nc.sync.dma_start(bv[:, q * 2:(q + 1) * 2, :],
                              xw1[q * 2:(q + 1) * 2].rearrange(
                                  "c p x -> p c x"))
        for q in range(n_chunks):
            t = bv[:, q, :]
            nc.tensor.matmul(
                psum[:], lhsT=t[:, :B], rhs=t[:, B:],
                start=(q == 0), stop=(q == n_chunks - 1),
            )
        out = opool.tile([B, JD], _bf16)
        nc.scalar.copy(out[:], psum[:])
        nc.sync.dma_start(sp[:], out[:])
    return nc


# ---------------------------------------------------------------------------
# Launches 2 & 3: one routing round.
# xV chunk lane table (64 chunks/launch): 0 = DVE direct (PSUM 1x),
# 1 = ACT copy + DVE bf16 2x, 2 = ACT copy + Pool mult.
def _mk_lanes(n_direct, n_actdve, n_actpool):
    assert n_direct + n_actdve + n_actpool == 256
    counts = {0: n_direct, 1: n_actdve, 2: n_actpool}
    done = {0: 0.0, 1: 0.0, 2: 0.0}
    order = []
    for k in range(256):
        # pick the mode furthest behind its proportional schedule
        m = max(counts, key=lambda mm: counts[mm] * (k + 1) / 256 - done[mm])
        order.append(m)
        done[m] += 1
    return order


XV_LANES = _mk_lanes(128, 38, 90)


N_VMM = J * 8            # 256 V-matmuls, 2 (j,ih)-pairs each
N_BLK = (N_VMM + 2) // 3  # 86 column blocks in wv3 (3 bands of 32 rows)


def build_l2(only=None, nu=None):
    nc = bass.Bass("TRN2", target_bir_lowering=False, debug=False)
    xt_d = nc.dram_tensor("xt", [128, IH * B], _bf16, kind="ExternalInput").ap()
    wv_d = nc.dram_tensor("wv", [64, 128 * 128], _bf16, kind="ExternalInput").ap()
    ws_d = nc.dram_tensor("ws", [128, J * IH * D], _bf16, kind="ExternalInput").ap()
    ot_d = nc.dram_tensor("ot4", [64, J * 512], _bf16, kind="ExternalInput").ap()
    e16_d = nc.dram_tensor("e16", [128, 16], _bf16, kind="ExternalInput").ap()
    r8_d = nc.dram_tensor("r8", [128, 8 * 128], _bf16, kind="ExternalInput").ap()
    id_d = nc.dram_tensor("idn", [128, 128], _bf16, kind="ExternalInput").ap()
    sp_d = nc.dram_tensor("spT", [16, J * B], _bf16, kind="ExternalOutput").ap()

    mult = mybir.AluOpType.mult

    with ExitStack() as ctx:
        tc = ctx.enter_context(tile.TileContext(nc))
        cpool = ctx.enter_context(tc.tile_pool(name="const", bufs=1))
        epool = ctx.enter_context(tc.tile_pool(name="eall", bufs=1))
        xvpool = ctx.enter_context(tc.tile_pool(name="xv", bufs=8))
        ypool = ctx.enter_context(tc.tile_pool(name="y", bufs=8))

        xt = cpool.tile([128, IH * B], _bf16)
        e16 = cpool.tile([128, 16], _bf16)
        wv = cpool.tile([64, 128 * 128], _bf16)
        ot4 = cpool.tile([64, J * 512], _bf16)
        r8 = cpool.tile([128, 8 * 128], _bf16)
        idn = cpool.tile([128, 128], _bf16)
        ws = cpool.tile([128, J * IH * D], _bf16)
        # interleave quarter-loads of ot4/wv so the first V-matmuls can
        # start after ~1/4 of the upload; phase-B-only tensors go last.
        nc.sync.dma_start(e16[:], e16_d[:])
        OQ = J * 512 // 8
        WQ = 128 * 128 // 8
        for q in range(8):
            nc.sync.dma_start(ot4[:, q * OQ:(q + 1) * OQ],
                              ot_d[:, q * OQ:(q + 1) * OQ])
            nc.sync.dma_start(wv[:, q * WQ:(q + 1) * WQ],
                              wv_d[:, q * WQ:(q + 1) * WQ])
            if q == 0:
                nc.sync.dma_start(xt[:], xt_d[:])
        nc.sync.dma_start(idn[:], id_d[:])
        nc.sync.dma_start(r8[:], r8_d[:])
        for q in range(2):
            HQ = J * IH * D // 2
            nc.sync.dma_start(ws[:, q * HQ:(q + 1) * HQ],
                              ws_d[:, q * HQ:(q + 1) * HQ])

        spT = epool.tile([16, J * B], _bf16)
        e_all = epool.tile([128, J * 256], _bf16)      # [b, (j, ih, il)]
        ets_all = epool.tile([128, J * 256], _bf16)    # [(v,il), (j, h, b)]
        xz = epool.tile([128, IH * B], _bf16)          # [q, (ih, b)]

        zp = epool.tile([128, 10 * 256], _f32)         # Z partials
        zacc = epool.tile([128, 256], _f32)            # running Z sum

        # PSUM-chunk multiply lanes: 0 = DVE direct from PSUM (1x),
        # 1 = ACT copy to SBUF bf16 then DVE mult (2x), 2 = ACT copy
        # then Pool mult. Cycled per chunk ([128, 1024]).
        lane_k = [0]

        def chunk_mult(out_t, x_ap, v_tile):
            lane = XV_LANES[lane_k[0] % len(XV_LANES)]
            lane_k[0] += 1
            if lane == 0:
                nc.vector.tensor_tensor(out_t[:], x_ap, v_tile[:], op=mult)
            else:
                vc = xvpool.tile([128, 512], _bf16, name="vc")
                nc.scalar.copy(vc[:], v_tile[:])
                eng = nc.vector if lane == 1 else nc.gpsimd
                eng.tensor_tensor(out_t[:], x_ap, vc[:], op=mult)

        # ---------------- phase A: e = exp(O . u_hat) --------------------
        with tc.tile_pool(name="vps", bufs=5, space="PSUM") as vpool, \
             tc.tile_pool(name="gps", bufs=2, space="PSUM") as gpool, \
             tc.tile_pool(name="aetp", bufs=1, space="PSUM") as aetpool:
            va, xva, ga = {}, {}, {}

            def a_v(u):
                j, qq = divmod(u, 4)
                v = vpool.tile([128, 512], _f32, name="v")
                nc.tensor.matmul(
                    v[:],
                    lhsT=wv[:, u * 128:(u + 1) * 128],
                    rhs=ot4[:, j * 512:(j + 1) * 512],
                    start=True, stop=True,
                )
                va[u] = v

            def a_xv(u):
                j, qq = divmod(u, 4)
                xv = xvpool.tile([128, 512], _bf16, name="xv")
                chunk_mult(xv, xt[:, qq * 512:(qq + 1) * 512], va[u])
                xva[u] = xv

            def a_g(u):
                j, qq = divmod(u, 4)
                jp = j // 2
                if qq == 0 and j % 2 == 0:
                    ga[jp] = gpool.tile([128, 512], _f32, name="g")
                g = ga[jp]
                xv = xva[u]
                for ii in range(4):
                    ih = qq * 4 + ii
                    col = (j % 2) * 256 + ih * 16
                    nc.tensor.matmul(
                        g[:, col:col + 16],
                        lhsT=xv[:, ii * 128:(ii + 1) * 128],
                        rhs=e16[:],
                        start=True, stop=True,
                    )
                del va[u], xva[u]

            def a_exp(jp):
                # exp for the j-pair (2*jp, 2*jp+1) in one ACT op
                nc.scalar.activation(
                    e_all[:, jp * 512:(jp + 1) * 512], ga[jp][:],
                    mybir.ActivationFunctionType.Exp,
                )
                del ga[jp]

            def a_ets(jp):
                et = aetpool.tile([128, 512], _bf16, name="et")
                for h in range(4):
                    nc.tensor.transpose(
                        et[:, h * 128:(h + 1) * 128],
                        e_all[:, jp * 512 + h * 128:
                              jp * 512 + (h + 1) * 128],
                        idn[:])
                nc.scalar.copy(ets_all[:, jp * 512:(jp + 1) * 512], et[:])

            LOOK_A = 9
            n_u = 4 * J if only != 'B' else 0
            n_u = min(n_u, nu) if nu else n_u
            for u in range(min(LOOK_A, n_u)):
                a_v(u)
            for u in range(n_u):
                if u + LOOK_A < n_u:
                    a_v(u + LOOK_A)
                a_xv(u)
                a_g(u)
                if u >= 15 and u % 8 == 7:
                    jp = u // 8 - 1
                    a_exp(jp)
                    a_ets(jp)
                    if jp % 2 == 1:
                        k4 = jp // 2
                        jj = 4 * k4 + 3
                        nc.vector.reduce_sum(
                            zp[:, k4 * 256:(k4 + 1) * 256],
                            e_all[:, (jj - 3) * 256:(jj + 1) * 256]
                            .rearrange("b (j i) -> b i j", j=4, i=256),
                            axis=mybir.AxisListType.X,
                        )
                        if k4 == 1:
                            nc.vector.tensor_add(
                                zacc[:], zp[:, :256], zp[:, 256:512])
                        elif k4 > 1:
                            nc.vector.tensor_add(
                                zacc[:], zacc[:],
                                zp[:, k4 * 256:(k4 + 1) * 256])
            if n_u == 4 * J:
                nc.vector.reduce_sum(
                    zp[:, 8 * 256:9 * 256],
                    e_all[:, 28 * 256:30 * 256]
                    .rearrange("b (j i) -> b i j", j=2, i=256),
                    axis=mybir.AxisListType.X,
                )
                nc.vector.tensor_add(
                    zacc[:], zacc[:], zp[:, 8 * 256:9 * 256])
            if n_u:
                a_exp(n_u // 8 - 1)
                a_ets(n_u // 8 - 1)
                if n_u == 4 * J:
                    nc.vector.reduce_sum(
                        zp[:, 9 * 256:10 * 256],
                        e_all[:, 30 * 256:32 * 256]
                        .rearrange("b (j i) -> b i j", j=2, i=256),
                        axis=mybir.AxisListType.X,
                    )

        # ---------------- softmax z-chain -> xz ---------------------------
        with tc.tile_pool(name="zps", bufs=1, space="PSUM") as zpool, \
             tc.tile_pool(name="zsb", bufs=1) as zsbuf:
            if only != 'A2':
                nc.vector.tensor_add(
                    zacc[:], zacc[:], zp[:, 9 * 256:10 * 256])
                zr16 = zsbuf.tile([128, 256], _bf16)
                with nc.allow_low_precision(reason="1/Z in bf16 is ample"):
                    nc.vector.reciprocal(zr16[:], zacc[:])
                zt = zpool.tile([128, 256], _bf16)
                for h in range(2):
                    nc.tensor.transpose(
                        zt[:, h * 128:(h + 1) * 128],
                        zr16[:, h * 128:(h + 1) * 128], idn[:])
                zr_ts = zsbuf.tile([128, 256], _bf16)
                nc.scalar.copy(zr_ts[:], zt[:])
                for half in range(2):
                    zps = zpool.tile([128, 1024], _f32, name="zpsh")
                    for hh in range(8):
                        ih = half * 8 + hh
                        nc.tensor.matmul(
                            zps[:, hh * 128:(hh + 1) * 128],
                            lhsT=r8[:, (ih % 8) * 128:(ih % 8 + 1) * 128],
                            rhs=zr_ts[:, (ih // 8) * 128:
                                      (ih // 8 + 1) * 128],
                            start=True, stop=True,
                        )
                    nc.vector.tensor_tensor(
                        xz[:, half * 1024:(half + 1) * 1024],
                        xt[:, half * 1024:(half + 1) * 1024],
                        zps[:], op=mult)

        # ---------------- phase B: s^T = sum_i c . u_hat ------------------
        with tc.tile_pool(name="wps", bufs=6, space="PSUM") as wpool, \
             tc.tile_pool(name="sps", bufs=2, space="PSUM") as sppool:
            w_t, y_t, s_t = {}, {}, {}

            def b_rep(u):
                j, qq = divmod(u, 4)
                w = wpool.tile([128, 512], _f32, name="w")
                for ii in range(4):
                    ih = qq * 4 + ii
                    nc.tensor.matmul(
                        w[:, ii * 128:(ii + 1) * 128],
                        lhsT=r8[:, (ih % 8) * 128:(ih % 8 + 1) * 128],
                        rhs=ets_all[:, j * 256 + (ih // 8) * 128:
                                    j * 256 + (ih // 8 + 1) * 128],
                        start=True, stop=True,
                    )
                w_t[u] = w

            def b_y(u):
                j, qq = divmod(u, 4)
                y = ypool.tile([128, 512], _bf16, name="y")
                chunk_mult(y, xz[:, qq * 512:(qq + 1) * 512], w_t[u])
                y_t[u] = y
                del w_t[u]

            def b_s(u):
                j, qq = divmod(u, 4)
                jp = j // 2
                if qq == 0 and j % 2 == 0:
                    s_t[jp] = sppool.tile([16, 2 * B], _f32, name="sp")
                s_ps = s_t[jp][:, (j % 2) * B:(j % 2 + 1) * B]
                y = y_t[u]
                for ii in range(4):
                    ih = qq * 4 + ii
                    nc.tensor.matmul(
                        s_ps,
                        lhsT=ws[:, j * 256 + ih * 16:
                                j * 256 + (ih + 1) * 16],
                        rhs=y[:, ii * 128:(ii + 1) * 128],
                        start=(ih == 0), stop=(ih == 15),
                    )
                del y_t[u]

            def b_scopy(jp):
                nc.scalar.copy(spT[:, jp * 2 * B:(jp + 1) * 2 * B],
                               s_t[jp][:])
                del s_t[jp]

            LOOK_B = 8
            skip_b = only in ('A', 'A2')
            n_u = 0 if skip_b else 4 * J
            for u in range(min(LOOK_B, n_u)):
                b_rep(u)

            for u in range(n_u):
                if u + LOOK_A < n_u:
                    a_v(u + LOOK_A)
                a_xv(u)
                a_g(u)
                if u >= 15 and u % 8 == 7:
                    jp = u // 8 - 1
                    a_exp(jp)
                    a_ets(jp)
                    if jp % 2 == 1:
                        k4 = jp // 2
                        jj = 4 * k4 + 3
                        nc.vector.reduce_sum(
                            zp[:, k4 * 256:(k4 + 1) * 256],
                            e_all[:, (jj - 3) * 256:(jj + 1) * 256]
                            .rearrange("b (j i) -> b i j", j=4, i=256),
                            axis=mybir.AxisListType.X,
                        )
                        if k4 == 1:
                            nc.vector.tensor_add(
                                zacc[:], zp[:, :256], zp[:, 256:512])
                        elif k4 > 1:
                            nc.vector.tensor_add(
                                zacc[:], zacc[:],
                                zp[:, k4 * 256:(k4 + 1) * 256])
            if n_u == 4 * J:
                nc.vector.reduce_sum(
                    zp[:, 8 * 256:9 * 256],
                    e_all[:, 28 * 256:30 * 256]
                    .rearrange("b (j i) -> b i j", j=2, i=256),
                    axis=mybir.AxisListType.X,
                )
                nc.vector.tensor_add(
                    zacc[:], zacc[:], zp[:, 8 * 256:9 * 256])
            if n_u:
                a_exp(n_u // 8 - 1)
                a_ets(n_u // 8 - 1)
                if n_u == 4 * J:
                    nc.vector.reduce_sum(
                        zp[:, 9 * 256:10 * 256],
                        e_all[:, 30 * 256:32 * 256]
                        .rearrange("b (j i) -> b i j", j=2, i=256),
                        axis=mybir.AxisListType.X,
                    )

        # -------- phase B (with the softmax z-chain overlapped) ----------
        # The b_rep R-matmuls only need ets_all/r8, so a few are emitted
        # before the z-chain; the PE chews on them while DVE computes
        # Z -> 1/Z -> xz.
        with tc.tile_pool(name="wps", bufs=3, space="PSUM") as wpool, \
             tc.tile_pool(name="sps", bufs=2, space="PSUM") as sppool, \
             tc.tile_pool(name="zps", bufs=1, space="PSUM") as zpool, \
             tc.tile_pool(name="zsb", bufs=1) as zsbuf:
            w_t, y_t, s_t = {}, {}, {}

            def b_rep(u):
                j, qq = divmod(u, 4)
                w = wpool.tile([128, 512], _f32, name="w")
                for ii in range(4):
                    ih = qq * 4 + ii
                    nc.tensor.matmul(
                        w[:, ii * 128:(ii + 1) * 128],
                        lhsT=r8[:, (ih % 8) * 128:(ih % 8 + 1) * 128],
                        rhs=ets_all[:, j * 256 + (ih // 8) * 128:
                                    j * 256 + (ih // 8 + 1) * 128],
                        start=True, stop=True,
                    )
                w_t[u] = w

            def b_y(u):
                j, qq = divmod(u, 4)
                y = ypool.tile([128, 512], _bf16, name="y")
                chunk_mult(y, xz[:, qq * 512:(qq + 1) * 512], w_t[u])
                y_t[u] = y
                del w_t[u]

            def b_s(u):
                j, qq = divmod(u, 4)
                jp = j // 2
                if qq == 0 and j % 2 == 0:
                    s_t[jp] = sppool.tile([16, 2 * B], _f32, name="sp")
                s_ps = s_t[jp][:, (j % 2) * B:(j % 2 + 1) * B]
                y = y_t[u]
                for ii in range(4):
                    ih = qq * 4 + ii
                    nc.tensor.matmul(
                        s_ps,
                        lhsT=ws[:, j * 256 + ih * 16:
                                j * 256 + (ih + 1) * 16],
                        rhs=y[:, ii * 128:(ii + 1) * 128],
                        start=(ih == 0), stop=(ih == 15),
                    )
                del y_t[u]

            def b_scopy(jp):
                nc.scalar.copy(spT[:, jp * 2 * B:(jp + 1) * 2 * B],
                               s_t[jp][:])
                del s_t[jp]

            LOOK_B = 3
            skip_b = only in ('A', 'A2')
            n_u = 0 if skip_b else 4 * J
            for u in range(min(LOOK_B, n_u)):
                b_rep(u)

            if only != 'A2':
                nc.vector.tensor_add(
                    zacc[:], zacc[:], zp[:, 9 * 256:10 * 256])
                zr16 = zsbuf.tile([128, 256], _bf16)
                with nc.allow_low_precision(reason="1/Z in bf16 is ample"):
                    nc.vector.reciprocal(zr16[:], zacc[:])
                zt = zpool.tile([128, 256], _bf16)
                for h in range(2):
                    nc.tensor.transpose(
                        zt[:, h * 128:(h + 1) * 128],
                        zr16[:, h * 128:(h + 1) * 128], idn[:])
                zr_ts = zsbuf.tile([128, 256], _bf16)
                nc.scalar.copy(zr_ts[:], zt[:])
                for half in range(2):
                    zps = zpool.tile([128, 1024], _f32, name="zpsh")
                    for hh in range(8):
                        ih = half * 8 + hh
                        nc.tensor.matmul(
                            zps[:, hh * 128:(hh + 1) * 128],
                            lhsT=r8[:, (ih % 8) * 128:(ih % 8 + 1) * 128],
                            rhs=zr_ts[:, (ih // 8) * 128:
                                      (ih // 8 + 1) * 128],
                            start=True, stop=True,
                        )
                    nc.vector.tensor_tensor(
                        xz[:, half * 1024:(half + 1) * 1024],
                        xt[:, half * 1024:(half + 1) * 1024],
                        zps[:], op=mult)

            for u in range(n_u):
                if u + LOOK_B < n_u:
                    b_rep(u + LOOK_B)
                b_y(u)
                b_s(u)
                if u >= 15 and u % 8 == 7:
                    jp = u // 8 - 1
                    b_scopy(jp)
                    if jp in (3, 7, 11, 14):
                        q0 = (jp - 3) * 2 * B if jp != 14 else 22 * B
                        qw = 8 * B if jp != 14 else 8 * B
                        nc.sync.dma_start(sp_d[:, q0:q0 + qw],
                                          spT[:, q0:q0 + qw])
            if n_u:
                b_scopy(J // 2 - 1)

        if only == 'A':
            nc.gpsimd.memset(spT[:], 0.0)
            nc.sync.dma_start(sp_d[:], spT[:])
        else:
            nc.sync.dma_start(sp_d[:, 30 * B:], spT[:, 30 * B:])
    return nc


# ---------------------------------------------------------------------------
# Host glue
def _squash(s):
    v = s.reshape(B, J, D).astype(np.float32)
    s2 = np.sum(np.square(v), axis=-1, keepdims=True)
    scale = s2 / (1.0 + s2) / np.sqrt(s2 + EPS)
    return (scale * v).astype(np.float32)


_cache = {}


def _get_nc(name):
    if name not in _cache:
        _cache[name] = build_l1() if name == "l1" else build_l2()
    return _cache[name]


def _prep_inputs(x, W):
    """Per-core host-side re-layouts (cheap numpy transposes + bf16 cast)."""
    e16 = np.zeros((128, 16), np.float32)
    e16[np.arange(128), np.arange(128) % 16] = 1.0
    e16 = e16.astype(BF16)
    r8 = np.zeros((128, 8 * 128), np.float32)
    for v in range(8):
        r8[v * 16 + np.arange(128) % 16, v * 128 + np.arange(128)] = 1.0
    r8 = r8.astype(BF16)
    idn = np.eye(128, dtype=np.float32).astype(BF16)

    per_core = []
    for c in range(N_CORES):
        sl = slice(c * I_LOC, (c + 1) * I_LOC)
        xc = x[:, sl, :]                                   # [B, I_LOC, P]
        wc = W[:, sl, :, :]                                # [J, I_LOC, D, P]
        # L1 interleaved chunks [(i,p)-part, B | JD]
        xp = np.ascontiguousarray(
            xc.transpose(1, 2, 0).reshape(I_LOC * P, B))
        wt = np.ascontiguousarray(
            wc.transpose(1, 3, 0, 2).reshape(I_LOC * P, JD))
        n_chunks = (I_LOC * P) // 128
        xw1 = np.empty((n_chunks, 128, B + JD), np.float32)
        xw1[:, :, :B] = xp.reshape(n_chunks, 128, B)
        xw1[:, :, B:] = wt.reshape(n_chunks, 128, JD)
        # x_t [q=(p,il), (ih, b)]
        x4 = xc.reshape(B, IH, IL, P)                      # b, ih, il, p
        x_t = np.ascontiguousarray(
            x4.transpose(3, 2, 1, 0).reshape(128, IH * B))
        # wv5: V-matmul m (= j*4 + zz) covers pairs ih = zz*4 + k,
        # k in [0,4): lhsT block [64, 128], slot k rows [16k, 16k+16).
        w5 = wc.reshape(J, IH, IL, D, P)                   # j, ih, il, d, p
        wblk = w5.transpose(0, 1, 3, 4, 2).reshape(J, IH, D, 128)  # [d,(p,il)]
        wv = np.zeros((64, 128 * 128), np.float32)
        for m in range(128):
            j, zz = divmod(m, 4)
            for k in range(4):
                ih = zz * 4 + k
                wv[16 * k:16 * (k + 1),
                   m * 128:(m + 1) * 128] = wblk[j, ih]
        # ws [q, (j, ih, d)]
        ws_ = np.ascontiguousarray(
            w5.transpose(4, 2, 0, 1, 3).reshape(128, J * IH * D))
        per_core.append({
            "xw1": xw1.astype(BF16),
            "xt": x_t.astype(BF16),
            "wv": wv.astype(BF16),
            "ws": ws_.astype(BF16),
            "e16": e16, "r8": r8, "idn": idn,
        })
    return per_core


def _ot4_layout(O):
    """O [B, J, D] f32 -> ot5 [64, J*512] bf16: per j a [64, 512] block
    diagonal with O_j^T at rows [16k,16k+16) x cols [128k,128(k+1))."""
    ojt = O.transpose(1, 2, 0)                      # [J, D, B]
    out = np.zeros((64, J * 512), np.float32)
    for k in range(4):
        out[16 * k:16 * (k + 1),
            np.arange(J)[:, None] * 512 + 128 * k + np.arange(128)[None, :]
            ] = ojt.transpose(1, 0, 2)
    return np.ascontiguousarray(out).astype(BF16)


def _run(nc, in_maps, **kw):
    return run_bass_kernel_spmd(nc, in_maps, list(range(N_CORES)), **kw)


def kernel(x, W, _collect_times=None):
    x = np.asarray(x, dtype=np.float32)
    W = np.asarray(W, dtype=np.float32)
    pc = _prep_inputs(x, W)

    nc1 = _get_nc("l1")
    nc2 = _get_nc("l2")

    r1 = _run(nc1, [{"xw1": p["xw1"]} for p in pc])
    s0 = np.sum([np.asarray(r1.results[c]["sp"], dtype=np.float32)
                 for c in range(N_CORES)], axis=0)
    s0 *= (1.0 / J)
    out0 = _squash(s0)
    O1 = out0.reshape(B, J, D)

    def l2_maps(Oacc):
        ot4 = _ot4_layout(Oacc)
        return [{"xt": p["xt"], "wv": p["wv"], "ws": p["ws"], "ot4": ot4,
                 "e16": p["e16"], "r8": p["r8"], "idn": p["idn"]}
                for p in pc]

    r2 = _run(nc2, l2_maps(O1))
    # spT [16, J*B] -> s[b, j, d]
    s1 = np.sum([np.asarray(r2.results[c]["spT"], dtype=np.float32)
                 for c in range(N_CORES)], axis=0)
    s1 = s1.reshape(D, J, B).transpose(2, 1, 0)
    out1 = _squash(s1)
    O2 = (out0.reshape(B, J, D) + out1.reshape(B, J, D))

    r3 = _run(nc2, l2_maps(O2))
    s2 = np.sum([np.asarray(r3.results[c]["spT"], dtype=np.float32)
                 for c in range(N_CORES)], axis=0)
    s2 = s2.reshape(D, J, B).transpose(2, 1, 0)
    out2 = _squash(s2)

    if _collect_times is not None:
        for r in (r1, r2, r3):
            _collect_times.append(r.exec_time_ns)
    return out2


# revision 3
# speedup vs baseline: 1.0104x; 1.0073x over previous
"""CapsuleLayer dynamic-routing kernel v2 for 8 Trainium2 NeuronCores.

Strategy vs v1: move every reduction/replication onto the PE (whose cost
is only proportional to output columns) and keep just the two irreducible
broadcast-multiplies per (j, round) on DVE/GpSimd, in bf16.

Layout (per core, I_LOC = 256): i = ih*16 + il, SBUF partitions
q = p*16 + il. Then per (j, ih) everything is 128x128 tiles:
  V-matmul:  V[q, b]   = sum_d Wv[d, q] * O^T[d, b]        (PE)
  xV mult:   xV = x_t (.) V                                 (DVE/Pool)
  g-reduce:  g[b, il]  = xV^T @ E16   (sums over p)         (PE)
  exp:       e = exp(g)                                     (ACT, from PSUM)
  softmax Z: strided reduce over j + reciprocal             (DVE)
  replicate: w[q, b] = R_v^T @ e_t    (broadcast over p)    (PE)
  y mult:    y = xz (.) w   (xz = x * 1/Z, replicated)      (DVE/Pool)
  s-matmul:  s^T[d, b] += Ws^T @ y   (accum over ih)        (PE)

3 launches (round 0 uniform-c + two routing rounds); the tiny
squash/reduce between launches runs on the host, as in v1.
"""

import numpy as np
import ml_dtypes
from contextlib import ExitStack

import concourse.bass as bass
import concourse.mybir as mybir
from concourse import tile
from concourse.bass_utils import run_bass_kernel_spmd

BF16 = ml_dtypes.bfloat16

# ---------------------------------------------------------------------------
B, I, P = 128, 2048, 8
J, D = 32, 16
JD = J * D               # 512
N_CORES = 8
I_LOC = I // N_CORES     # 256
IH = IL = 16
EPS = 1e-7

_f32 = mybir.dt.float32
_bf16 = mybir.dt.bfloat16


# ---------------------------------------------------------------------------
# Walrus compat: this toolchain rejects sync waits on InstDrain and >2 on
# InstEventSemaphore. Emit the waits as standalone nops before the drain.
def _apply_tile_compat():
    from concourse.vector_clock import ScopedClock

    def _strip_waits(inst):
        si = inst.sync_info
        if not si or not si.on_wait:
            return []
        waits = list(si.on_wait)
        si.on_wait = []
        inst.sync_info = si
        return waits

    def _nop_with_wait(eng, w):
        nop = eng.nop(nofuse=True, hint="drain_wait_split")
        nsi = nop.ins.sync_info
        if nsi is None:
            nsi = mybir.SyncInfo(on_wait=[], on_update=[])
        nsi.on_wait = list(nsi.on_wait or []) + [w]
        nop.ins.sync_info = nsi

    def _patched_multi_engine_barrier(self, engines):
        for inst in bass._bass_rust._multi_engine_barrier_insts(
            self, list(engines)
        ):
            eng = self.engines[inst.engine]
            for w in _strip_waits(inst):
                _nop_with_wait(eng, w)
            eng.add_instruction(inst)

    def _patched_drain_and_barrier(self, tick_clock, wait_clock):
        nop_inst = self.nc.sync.nop(nofuse=True, hint="drain_wait_split")
        wait_clock.add_sem_waits(
            nop_inst.ins, ScopedClock({None: tick_clock.global_clock})
        )
        si = nop_inst.ins.sync_info
        if si and si.on_wait and len(si.on_wait) > 1:
            extra = list(si.on_wait[1:])
            si.on_wait = [si.on_wait[0]]
            nop_inst.ins.sync_info = si
            for w in extra:
                _nop_with_wait(self.nc.sync, w)
        self.nc.sync.drain()

        self.nc.all_engine_barrier()
        assert self.sems is not None
        popped = self.nc._tile_sem_poison_stack.pop()
        assert popped is self._sem_poison
        self.nc.clear_and_free_semaphores(list(self.sems.allocated().values()))

    _WAIT_CAPS = {"InstDrain": 0, "InstEventSemaphore": 2}
    _orig_add_instruction = tile.TileContext._add_instruction

    def _patched_add_instruction(self, inst):
        si = inst.sync_info
        cap = _WAIT_CAPS.get(type(inst).__name__, 1)
        if si and si.on_wait and len(si.on_wait) > cap:
            waits = list(si.on_wait)
            si.on_wait = waits[:cap]
            inst.sync_info = si
            for w in waits[cap:]:
                nop = mybir.InstNoOp(
                    name=f"I-{self.nc.next_id()}-waitspill", ins=[], outs=[]
                )
                nop.engine = inst.engine
                nop.sync_info = mybir.SyncInfo(on_wait=[w], on_update=[])
                _orig_add_instruction(self, nop)
        _orig_add_instruction(self, inst)

    bass.Bass.multi_engine_barrier = _patched_multi_engine_barrier
    tile.TileContext._drain_and_barrier = _patched_drain_and_barrier
    tile.TileContext._add_instruction = _patched_add_instruction


_apply_tile_compat()


# ---------------------------------------------------------------------------
# Launch 1: s0_part[b,(j,d)] = sum_{i local} u_hat[b,j,i,d], bf16 inputs.
def build_l1():
    nc = bass.Bass("TRN2", target_bir_lowering=False, debug=False)
    n_chunks = (I_LOC * P) // 128  # 16
    xw1 = nc.dram_tensor(
        "xw1", [n_chunks, 128, B + JD], _bf16, kind="ExternalInput").ap()
    sp = nc.dram_tensor("sp", [B, JD], _bf16, kind="ExternalOutput").ap()
    with ExitStack() as ctx:
        tc = ctx.enter_context(tile.TileContext(nc))
        xpool = ctx.enter_context(tc.tile_pool(name="xw1", bufs=1))
        ppool = ctx.enter_context(tc.tile_pool(name="ps", bufs=1, space="PSUM"))
        opool = ctx.enter_context(tc.tile_pool(name="o", bufs=1))
        psum = ppool.tile([B, JD], _f32)
        big = xpool.tile([128, n_chunks * (B + JD)], _bf16)
        bv = big[:].rearrange("p (c x) -> p c x", c=n_chunks, x=B + JD)
        # eighth-loads so the first matmuls start early
        for q in range(8):
            nc.sync.dma_start(bv[:, q * 2:(q + 1) * 2, :],
                              xw1[q * 2:(q + 1) * 2].rearrange(
                                  "c p x -> p c x"))
        for q in range(n_chunks):
            t = bv[:, q, :]
            nc.tensor.matmul(
                psum[:], lhsT=t[:, :B], rhs=t[:, B:],
                start=(q == 0), stop=(q == n_chunks - 1),
            )
        out = opool.tile([B, JD], _bf16)
        nc.scalar.copy(out[:], psum[:])
        nc.sync.dma_start(sp[:], out[:])
    return nc


# ---------------------------------------------------------------------------
# Launches 2 & 3: one routing round.
# xV chunk lane table (64 chunks/launch): 0 = DVE direct (PSUM 1x),
# 1 = ACT copy + DVE bf16 2x, 2 = ACT copy + Pool mult.
def _mk_lanes(n_direct, n_actdve, n_actpool):
    assert n_direct + n_actdve + n_actpool == 256
    counts = {0: n_direct, 1: n_actdve, 2: n_actpool}
    done = {0: 0.0, 1: 0.0, 2: 0.0}
    order = []
    for k in range(256):
        # pick the mode furthest behind its proportional schedule
        m = max(counts, key=lambda mm: counts[mm] * (k + 1) / 256 - done[mm])
        order.append(m)
        done[m] += 1
    return order


XV_LANES = _mk_lanes(128, 38, 90)


N_VMM = J * 8            # 256 V-matmuls, 2 (j,ih)-pairs each
N_BLK = (N_VMM + 2) // 3  # 86 column blocks in wv3 (3 bands of 32 rows)


def build_l2(only=None, nu=None):
    nc = bass.Bass("TRN2", target_bir_lowering=False, debug=False)
    xt_d = nc.dram_tensor("xt", [128, IH * B], _bf16, kind="ExternalInput").ap()
    wv_d = nc.dram_tensor("wv", [64, 128 * 128], _bf16, kind="ExternalInput").ap()
    ws_d = nc.dram_tensor("ws", [128, J * IH * D], _bf16, kind="ExternalInput").ap()
    ot_d = nc.dram_tensor("ot4", [64, J * 512], _bf16, kind="ExternalInput").ap()
    e16_d = nc.dram_tensor("e16", [128, 16], _bf16, kind="ExternalInput").ap()
    r8_d = nc.dram_tensor("r8", [128, 8 * 128], _bf16, kind="ExternalInput").ap()
    id_d = nc.dram_tensor("idn", [128, 128], _bf16, kind="ExternalInput").ap()
    sp_d = nc.dram_tensor("spT", [16, J * B], _bf16, kind="ExternalOutput").ap()

    mult = mybir.AluOpType.mult

    with ExitStack() as ctx:
        tc = ctx.enter_context(tile.TileContext(nc))
        cpool = ctx.enter_context(tc.tile_pool(name="const", bufs=1))
        epool = ctx.enter_context(tc.tile_pool(name="eall", bufs=1))
        xvpool = ctx.enter_context(tc.tile_pool(name="xv", bufs=8))
        ypool = ctx.enter_context(tc.tile_pool(name="y", bufs=8))

        xt = cpool.tile([128, IH * B], _bf16)
        e16 = cpool.tile([128, 16], _bf16)
        wv = cpool.tile([64, 128 * 128], _bf16)
        ot4 = cpool.tile([64, J * 512], _bf16)
        r8 = cpool.tile([128, 8 * 128], _bf16)
        idn = cpool.tile([128, 128], _bf16)
        ws = cpool.tile([128, J * IH * D], _bf16)
        # interleave quarter-loads of ot4/wv so the first V-matmuls can
        # start after ~1/4 of the upload; phase-B-only tensors go last.
        nc.sync.dma_start(e16[:], e16_d[:])
        OQ = J * 512 // 8
        WQ = 128 * 128 // 8
        for q in range(8):
            nc.sync.dma_start(ot4[:, q * OQ:(q + 1) * OQ],
                              ot_d[:, q * OQ:(q + 1) * OQ])
            nc.sync.dma_start(wv[:, q * WQ:(q + 1) * WQ],
                              wv_d[:, q * WQ:(q + 1) * WQ])
            if q == 0:
                nc.sync.dma_start(xt[:], xt_d[:])
        nc.sync.dma_start(idn[:], id_d[:])
        nc.sync.dma_start(r8[:], r8_d[:])
        for q in range(2):
            HQ = J * IH * D // 2
            nc.sync.dma_start(ws[:, q * HQ:(q + 1) * HQ],
                              ws_d[:, q * HQ:(q + 1) * HQ])

        spT = epool.tile([16, J * B], _bf16)
        e_all = epool.tile([128, J * 256], _bf16)      # [b, (j, ih, il)]
        ets_all = epool.tile([128, J * 256], _bf16)    # [(v,il), (j, h, b)]
        xz = epool.tile([128, IH * B], _bf16)          # [q, (ih, b)]

        zp = epool.tile([128, 10 * 256], _f32)         # Z partials
        zacc = epool.tile([128, 256], _f32)            # running Z sum

        # PSUM-chunk multiply lanes: 0 = DVE direct from PSUM (1x),
        # 1 = ACT copy to SBUF bf16 then DVE mult (2x), 2 = ACT copy
        # then Pool mult. Cycled per chunk ([128, 1024]).
        lane_k = [0]

        def chunk_mult(out_t, x_ap, v_tile):
            lane = XV_LANES[lane_k[0] % len(XV_LANES)]
            lane_k[0] += 1
            if lane == 0:
                nc.vector.tensor_tensor(out_t[:], x_ap, v_tile[:], op=mult)
            else:
                vc = xvpool.tile([128, 512], _bf16, name="vc")
                nc.scalar.copy(vc[:], v_tile[:])
                eng = nc.vector if lane == 1 else nc.gpsimd
                eng.tensor_tensor(out_t[:], x_ap, vc[:], op=mult)

        # ---------------- phase A: e = exp(O . u_hat) --------------------
        with tc.tile_pool(name="vps", bufs=5, space="PSUM") as vpool, \
             tc.tile_pool(name="gps", bufs=2, space="PSUM") as gpool, \
             tc.tile_pool(name="aetp", bufs=1, space="PSUM") as aetpool:
            va, xva, ga = {}, {}, {}

            def a_v(u):
                j, qq = divmod(u, 4)
                v = vpool.tile([128, 512], _f32, name="v")
                nc.tensor.matmul(
                    v[:],
                    lhsT=wv[:, u * 128:(u + 1) * 128],
                    rhs=ot4[:, j * 512:(j + 1) * 512],
                    start=True, stop=True,
                )
                va[u] = v

            def a_xv(u):
                j, qq = divmod(u, 4)
                xv = xvpool.tile([128, 512], _bf16, name="xv")
                chunk_mult(xv, xt[:, qq * 512:(qq + 1) * 512], va[u])
                xva[u] = xv

            def a_g(u):
                j, qq = divmod(u, 4)
                jp = j // 2
                if qq == 0 and j % 2 == 0:
                    ga[jp] = gpool.tile([128, 512], _f32, name="g")
                g = ga[jp]
                xv = xva[u]
                for ii in range(4):
                    ih = qq * 4 + ii
                    col = (j % 2) * 256 + ih * 16
                    nc.tensor.matmul(
                        g[:, col:col + 16],
                        lhsT=xv[:, ii * 128:(ii + 1) * 128],
                        rhs=e16[:],
                        start=True, stop=True,
                    )
                del va[u], xva[u]

            def a_exp(jp):
                # exp for the j-pair (2*jp, 2*jp+1) in one ACT op
                nc.scalar.activation(
                    e_all[:, jp * 512:(jp + 1) * 512], ga[jp][:],
                    mybir.ActivationFunctionType.Exp,
                )
                del ga[jp]

            def a_ets(jp):
                et = aetpool.tile([128, 512], _bf16, name="et")
                for h in range(4):
                    nc.tensor.transpose(
                        et[:, h * 128:(h + 1) * 128],
                        e_all[:, jp * 512 + h * 128:
                              jp * 512 + (h + 1) * 128],
                        idn[:])
                nc.scalar.copy(ets_all[:, jp * 512:(jp + 1) * 512], et[:])

            LOOK_A = 12
            n_u = 4 * J if only != 'B' else 0
            n_u = min(n_u, nu) if nu else n_u
            for u in range(min(LOOK_A, n_u)):
                a_v(u)
            for u in range(n_u):
                if u + LOOK_A < n_u:
                    a_v(u + LOOK_A)
                a_xv(u)
                a_g(u)
                if u >= 15 and u % 8 == 7:
                    jp = u // 8 - 1
                    a_exp(jp)
                    a_ets(jp)
                    if jp % 2 == 1:
                        k4 = jp // 2
                        jj = 4 * k4 + 3
                        nc.vector.reduce_sum(
                            zp[:, k4 * 256:(k4 + 1) * 256],
                            e_all[:, (jj - 3) * 256:(jj + 1) * 256]
                            .rearrange("b (j i) -> b i j", j=4, i=256),
                            axis=mybir.AxisListType.X,
                        )
                        if k4 == 1:
                            nc.vector.tensor_add(
                                zacc[:], zp[:, :256], zp[:, 256:512])
                        elif k4 > 1:
                            nc.vector.tensor_add(
                                zacc[:], zacc[:],
                                zp[:, k4 * 256:(k4 + 1) * 256])
            if n_u == 4 * J:
                nc.vector.reduce_sum(
                    zp[:, 8 * 256:9 * 256],
                    e_all[:, 28 * 256:30 * 256]
                    .rearrange("b (j i) -> b i j", j=2, i=256),
                    axis=mybir.AxisListType.X,
                )
                nc.vector.tensor_add(
                    zacc[:], zacc[:], zp[:, 8 * 256:9 * 256])
            if n_u:
                a_exp(n_u // 8 - 1)
                a_ets(n_u // 8 - 1)
                if n_u == 4 * J:
                    nc.vector.reduce_sum(
                        zp[:, 9 * 256:10 * 256],
                        e_all[:, 30 * 256:32 * 256]
                        .rearrange("b (j i) -> b i j", j=2, i=256),
                        axis=mybir.AxisListType.X,
                    )

        # ---------------- softmax z-chain -> xz ---------------------------
        with tc.tile_pool(name="zps", bufs=1, space="PSUM") as zpool, \
             tc.tile_pool(name="zsb", bufs=1) as zsbuf:
            if only != 'A2':
                nc.vector.tensor_add(
                    zacc[:], zacc[:], zp[:, 9 * 256:10 * 256])
                zr16 = zsbuf.tile([128, 256], _bf16)
                with nc.allow_low_precision(reason="1/Z in bf16 is ample"):
                    nc.vector.reciprocal(zr16[:], zacc[:])
                zt = zpool.tile([128, 256], _bf16)
                for h in range(2):
                    nc.tensor.transpose(
                        zt[:, h * 128:(h + 1) * 128],
                        zr16[:, h * 128:(h + 1) * 128], idn[:])
                zr_ts = zsbuf.tile([128, 256], _bf16)
                nc.scalar.copy(zr_ts[:], zt[:])
                for half in range(2):
                    zps = zpool.tile([128, 1024], _f32, name="zpsh")
                    for hh in range(8):
                        ih = half * 8 + hh
                        nc.tensor.matmul(
                            zps[:, hh * 128:(hh + 1) * 128],
                            lhsT=r8[:, (ih % 8) * 128:(ih % 8 + 1) * 128],
                            rhs=zr_ts[:, (ih // 8) * 128:
                                      (ih // 8 + 1) * 128],
                            start=True, stop=True,
                        )
                    nc.vector.tensor_tensor(
                        xz[:, half * 1024:(half + 1) * 1024],
                        xt[:, half * 1024:(half + 1) * 1024],
                        zps[:], op=mult)

        # ---------------- phase B: s^T = sum_i c . u_hat ------------------
        with tc.tile_pool(name="wps", bufs=6, space="PSUM") as wpool, \
             tc.tile_pool(name="sps", bufs=2, space="PSUM") as sppool:
            w_t, y_t, s_t = {}, {}, {}

            def b_rep(u):
                j, qq = divmod(u, 4)
                w = wpool.tile([128, 512], _f32, name="w")
                for ii in range(4):
                    ih = qq * 4 + ii
                    nc.tensor.matmul(
                        w[:, ii * 128:(ii + 1) * 128],
                        lhsT=r8[:, (ih % 8) * 128:(ih % 8 + 1) * 128],
                        rhs=ets_all[:, j * 256 + (ih // 8) * 128:
                                    j * 256 + (ih // 8 + 1) * 128],
                        start=True, stop=True,
                    )
                w_t[u] = w

            def b_y(u):
                j, qq = divmod(u, 4)
                y = ypool.tile([128, 512], _bf16, name="y")
                chunk_mult(y, xz[:, qq * 512:(qq + 1) * 512], w_t[u])
                y_t[u] = y
                del w_t[u]

            def b_s(u):
                j, qq = divmod(u, 4)
                jp = j // 2
                if qq == 0 and j % 2 == 0:
                    s_t[jp] = sppool.tile([16, 2 * B], _f32, name="sp")
                s_ps = s_t[jp][:, (j % 2) * B:(j % 2 + 1) * B]
                y = y_t[u]
                for ii in range(4):
                    ih = qq * 4 + ii
                    nc.tensor.matmul(
                        s_ps,
                        lhsT=ws[:, j * 256 + ih * 16:
                                j * 256 + (ih + 1) * 16],
                        rhs=y[:, ii * 128:(ii + 1) * 128],
                        start=(ih == 0), stop=(ih == 15),
                    )
                del y_t[u]

            def b_scopy(jp):
                nc.scalar.copy(spT[:, jp * 2 * B:(jp + 1) * 2 * B],
                               s_t[jp][:])
                del s_t[jp]

            LOOK_B = 10
            skip_b = only in ('A', 'A2')
            n_u = 0 if skip_b else 4 * J
            for u in range(min(LOOK_B, n_u)):
                b_rep(u)

            for u in range(n_u):
                if u + LOOK_A < n_u:
                    a_v(u + LOOK_A)
                a_xv(u)
                a_g(u)
                if u >= 15 and u % 8 == 7:
                    jp = u // 8 - 1
                    a_exp(jp)
                    a_ets(jp)
                    if jp % 2 == 1:
                        k4 = jp // 2
                        jj = 4 * k4 + 3
                        nc.vector.reduce_sum(
                            zp[:, k4 * 256:(k4 + 1) * 256],
                            e_all[:, (jj - 3) * 256:(jj + 1) * 256]
                            .rearrange("b (j i) -> b i j", j=4, i=256),
                            axis=mybir.AxisListType.X,
                        )
                        if k4 == 1:
                            nc.vector.tensor_add(
                                zacc[:], zp[:, :256], zp[:, 256:512])
                        elif k4 > 1:
                            nc.vector.tensor_add(
                                zacc[:], zacc[:],
                                zp[:, k4 * 256:(k4 + 1) * 256])
            if n_u == 4 * J:
                nc.vector.reduce_sum(
                    zp[:, 8 * 256:9 * 256],
                    e_all[:, 28 * 256:30 * 256]
                    .rearrange("b (j i) -> b i j", j=2, i=256),
                    axis=mybir.AxisListType.X,
                )
                nc.vector.tensor_add(
                    zacc[:], zacc[:], zp[:, 8 * 256:9 * 256])
            if n_u:
                a_exp(n_u // 8 - 1)
                a_ets(n_u // 8 - 1)
                if n_u == 4 * J:
                    nc.vector.reduce_sum(
                        zp[:, 9 * 256:10 * 256],
                        e_all[:, 30 * 256:32 * 256]
                        .rearrange("b (j i) -> b i j", j=2, i=256),
                        axis=mybir.AxisListType.X,
                    )

        # -------- phase B (with the softmax z-chain overlapped) ----------
        # The b_rep R-matmuls only need ets_all/r8, so a few are emitted
        # before the z-chain; the PE chews on them while DVE computes
        # Z -> 1/Z -> xz.
        with tc.tile_pool(name="wps", bufs=3, space="PSUM") as wpool, \
             tc.tile_pool(name="sps", bufs=2, space="PSUM") as sppool, \
             tc.tile_pool(name="zps", bufs=1, space="PSUM") as zpool, \
             tc.tile_pool(name="zsb", bufs=1) as zsbuf:
            w_t, y_t, s_t = {}, {}, {}

            def b_rep(u):
                j, qq = divmod(u, 4)
                w = wpool.tile([128, 512], _f32, name="w")
                for ii in range(4):
                    ih = qq * 4 + ii
                    nc.tensor.matmul(
                        w[:, ii * 128:(ii + 1) * 128],
                        lhsT=r8[:, (ih % 8) * 128:(ih % 8 + 1) * 128],
                        rhs=ets_all[:, j * 256 + (ih // 8) * 128:
                                    j * 256 + (ih // 8 + 1) * 128],
                        start=True, stop=True,
                    )
                w_t[u] = w

            def b_y(u):
                j, qq = divmod(u, 4)
                y = ypool.tile([128, 512], _bf16, name="y")
                chunk_mult(y, xz[:, qq * 512:(qq + 1) * 512], w_t[u])
                y_t[u] = y
                del w_t[u]

            def b_s(u):
                j, qq = divmod(u, 4)
                jp = j // 2
                if qq == 0 and j % 2 == 0:
                    s_t[jp] = sppool.tile([16, 2 * B], _f32, name="sp")
                s_ps = s_t[jp][:, (j % 2) * B:(j % 2 + 1) * B]
                y = y_t[u]
                for ii in range(4):
                    ih = qq * 4 + ii
                    nc.tensor.matmul(
                        s_ps,
                        lhsT=ws[:, j * 256 + ih * 16:
                                j * 256 + (ih + 1) * 16],
                        rhs=y[:, ii * 128:(ii + 1) * 128],
                        start=(ih == 0), stop=(ih == 15),
                    )
                del y_t[u]

            def b_scopy(jp):
                nc.scalar.copy(spT[:, jp * 2 * B:(jp + 1) * 2 * B],
                               s_t[jp][:])
                del s_t[jp]

            LOOK_B = 3
            skip_b = only in ('A', 'A2')
            n_u = 0 if skip_b else 4 * J
            for u in range(min(LOOK_B, n_u)):
                b_rep(u)

            if only != 'A2':
                nc.vector.tensor_add(
                    zacc[:], zacc[:], zp[:, 9 * 256:10 * 256])
                zr16 = zsbuf.tile([128, 256], _bf16)
                with nc.allow_low_precision(reason="1/Z in bf16 is ample"):
                    nc.vector.reciprocal(zr16[:], zacc[:])
                zt = zpool.tile([128, 256], _bf16)
                for h in range(2):
                    nc.tensor.transpose(
                        zt[:, h * 128:(h + 1) * 128],
                        zr16[:, h * 128:(h + 1) * 128], idn[:])
                zr_ts = zsbuf.tile([128, 256], _bf16)
                nc.scalar.copy(zr_ts[:], zt[:])
                for half in range(2):
                    zps = zpool.tile([128, 1024], _f32, name="zpsh")
                    for hh in range(8):
                        ih = half * 8 + hh
                        nc.tensor.matmul(
                            zps[:, hh * 128:(hh + 1) * 128],
                            lhsT=r8[:, (ih % 8) * 128:(ih % 8 + 1) * 128],
                            rhs=zr_ts[:, (ih // 8) * 128:
                                      (ih // 8 + 1) * 128],
                            start=True, stop=True,
                        )
                    nc.vector.tensor_tensor(
                        xz[:, half * 1024:(half + 1) * 1024],
                        xt[:, half * 1024:(half + 1) * 1024],
                        zps[:], op=mult)

            for u in range(n_u):
                if u + LOOK_B < n_u:
                    b_rep(u + LOOK_B)
                b_y(u)
                b_s(u)
                if u >= 15 and u % 8 == 7:
                    jp = u // 8 - 1
                    b_scopy(jp)
                    if jp in (3, 7, 11, 14):
                        q0 = (jp - 3) * 2 * B if jp != 14 else 22 * B
                        qw = 8 * B if jp != 14 else 8 * B
                        nc.sync.dma_start(sp_d[:, q0:q0 + qw],
                                          spT[:, q0:q0 + qw])
            if n_u:
                b_scopy(J // 2 - 1)

        if only == 'A':
            nc.gpsimd.memset(spT[:], 0.0)
            nc.sync.dma_start(sp_d[:], spT[:])
        else:
            nc.sync.dma_start(sp_d[:, 30 * B:], spT[:, 30 * B:])
    return nc


# ---------------------------------------------------------------------------
# Host glue
def _squash(s):
    v = s.reshape(B, J, D).astype(np.float32)
    s2 = np.sum(np.square(v), axis=-1, keepdims=True)
    scale = s2 / (1.0 + s2) / np.sqrt(s2 + EPS)
    return (scale * v).astype(np.float32)


_cache = {}


def _get_nc(name):
    if name not in _cache:
        _cache[name] = build_l1() if name == "l1" else build_l2()
    return _cache[name]


def _prep_inputs(x, W):
    """Per-core host-side re-layouts (cheap numpy transposes + bf16 cast)."""
    e16 = np.zeros((128, 16), np.float32)
    e16[np.arange(128), np.arange(128) % 16] = 1.0
    e16 = e16.astype(BF16)
    r8 = np.zeros((128, 8 * 128), np.float32)
    for v in range(8):
        r8[v * 16 + np.arange(128) % 16, v * 128 + np.arange(128)] = 1.0
    r8 = r8.astype(BF16)
    idn = np.eye(128, dtype=np.float32).astype(BF16)

    per_core = []
    for c in range(N_CORES):
        sl = slice(c * I_LOC, (c + 1) * I_LOC)
        xc = x[:, sl, :]                                   # [B, I_LOC, P]
        wc = W[:, sl, :, :]                                # [J, I_LOC, D, P]
        # L1 interleaved chunks [(i,p)-part, B | JD]
        xp = np.ascontiguousarray(
            xc.transpose(1, 2, 0).reshape(I_LOC * P, B))
        wt = np.ascontiguousarray(
            wc.transpose(1, 3, 0, 2).reshape(I_LOC * P, JD))
        n_chunks = (I_LOC * P) // 128
        xw1 = np.empty((n_chunks, 128, B + JD), np.float32)
        xw1[:, :, :B] = xp.reshape(n_chunks, 128, B)
        xw1[:, :, B:] = wt.reshape(n_chunks, 128, JD)
        # x_t [q=(p,il), (ih, b)]
        x4 = xc.reshape(B, IH, IL, P)                      # b, ih, il, p
        x_t = np.ascontiguousarray(
            x4.transpose(3, 2, 1, 0).reshape(128, IH * B))
        # wv5: V-matmul m (= j*4 + zz) covers pairs ih = zz*4 + k,
        # k in [0,4): lhsT block [64, 128], slot k rows [16k, 16k+16).
        w5 = wc.reshape(J, IH, IL, D, P)                   # j, ih, il, d, p
        wblk = w5.transpose(0, 1, 3, 4, 2).reshape(J, IH, D, 128)  # [d,(p,il)]
        wv = np.zeros((64, 128 * 128), np.float32)
        for m in range(128):
            j, zz = divmod(m, 4)
            for k in range(4):
                ih = zz * 4 + k
                wv[16 * k:16 * (k + 1),
                   m * 128:(m + 1) * 128] = wblk[j, ih]
        # ws [q, (j, ih, d)]
        ws_ = np.ascontiguousarray(
            w5.transpose(4, 2, 0, 1, 3).reshape(128, J * IH * D))
        per_core.append({
            "xw1": xw1.astype(BF16),
            "xt": x_t.astype(BF16),
            "wv": wv.astype(BF16),
            "ws": ws_.astype(BF16),
            "e16": e16, "r8": r8, "idn": idn,
        })
    return per_core


def _ot4_layout(O):
    """O [B, J, D] f32 -> ot5 [64, J*512] bf16: per j a [64, 512] block
    diagonal with O_j^T at rows [16k,16k+16) x cols [128k,128(k+1))."""
    ojt = O.transpose(1, 2, 0)                      # [J, D, B]
    out = np.zeros((64, J * 512), np.float32)
    for k in range(4):
        out[16 * k:16 * (k + 1),
            np.arange(J)[:, None] * 512 + 128 * k + np.arange(128)[None, :]
            ] = ojt.transpose(1, 0, 2)
    return np.ascontiguousarray(out).astype(BF16)


def _run(nc, in_maps, **kw):
    return run_bass_kernel_spmd(nc, in_maps, list(range(N_CORES)), **kw)


def kernel(x, W, _collect_times=None):
    x = np.asarray(x, dtype=np.float32)
    W = np.asarray(W, dtype=np.float32)
    pc = _prep_inputs(x, W)

    nc1 = _get_nc("l1")
    nc2 = _get_nc("l2")

    r1 = _run(nc1, [{"xw1": p["xw1"]} for p in pc])
    s0 = np.sum([np.asarray(r1.results[c]["sp"], dtype=np.float32)
                 for c in range(N_CORES)], axis=0)
    s0 *= (1.0 / J)
    out0 = _squash(s0)
    O1 = out0.reshape(B, J, D)

    def l2_maps(Oacc):
        ot4 = _ot4_layout(Oacc)
        return [{"xt": p["xt"], "wv": p["wv"], "ws": p["ws"], "ot4": ot4,
                 "e16": p["e16"], "r8": p["r8"], "idn": p["idn"]}
                for p in pc]

    r2 = _run(nc2, l2_maps(O1))
    # spT [16, J*B] -> s[b, j, d]
    s1 = np.sum([np.asarray(r2.results[c]["spT"], dtype=np.float32)
                 for c in range(N_CORES)], axis=0)
    s1 = s1.reshape(D, J, B).transpose(2, 1, 0)
    out1 = _squash(s1)
    O2 = (out0.reshape(B, J, D) + out1.reshape(B, J, D))

    r3 = _run(nc2, l2_maps(O2))
    s2 = np.sum([np.asarray(r3.results[c]["spT"], dtype=np.float32)
                 for c in range(N_CORES)], axis=0)
    s2 = s2.reshape(D, J, B).transpose(2, 1, 0)
    out2 = _squash(s2)

    if _collect_times is not None:
        for r in (r1, r2, r3):
            _collect_times.append(r.exec_time_ns)
    return out2
